# revision 20
# baseline (speedup 1.0000x reference)
"""Trainium2 Bass kernel for nn_Attn_32925219291574.

Math: reference computes softmax_s( v . (W @ [hidden; enc[b,s]] + b) ).
Split W = [Wh | We]. The hidden/bias part v.(Wh@hidden + b) is constant in s,
and softmax is shift-invariant, so the output is exactly
    softmax_s( enc[b,s,:] . u ),   u = v @ We    (We = W[:, H:2H])
`hidden` and `b` never affect the output. u (4 KB) is computed on the host
during input sharding, so the kernel is a pure stream over the 256 MiB
encoder_outputs tensor: per-row dot products, then a softmax per batch.

Engine budget: the fused multiply+row-sum (TensorScalarPtr/accum_out) runs
only in the DVE's 1x perf mode (~1.5us per [128,1024] fp32 tile -> ~96us for
all 64 tiles, which made DVE the baseline's bottleneck, above the ~94us HBM
streaming floor). So the work is split into two pipelines:
  A (20 tiles, fp32): fused STT on DVE, exact.
  B (44 tiles, fp16): chunk is cast f32->fp16 during the DMA (SWDGE/gpsimd
     queue, the only engine that can cast), DVE does a plain tensor_tensor
     multiply (2x_1p mode, ~0.9us), and the ACT engine row-sums the product
     via activation(Copy, accum_out) (~1.5us) in parallel.
fp16 quantization of enc/u perturbs the logits by ~0.01 (measured softmax
rel err ~1e-3, budget 2e-2). DVE ~72us, ACT ~70us, both under the DMA floor.

The softmax uses a fixed shift C = 4.5*||u|| instead of the data max
(scores ~ N(0, ~1.2||u||) since enc is unit-normal; exp(max-C) can neither
overflow nor all-underflow within ~8 sigma) — this removes the max
reduction/transpose/broadcast from the kernel tail entirely.

Sharding: data-parallel over batch B=16 -> 2 batches per core, no cross-core
communication. fp32 chunks stream on the two HWDGE rings (SP/ACT); fp16
chunks + u load + output stores ride the SWDGE (gpsimd) queue.
"""

import numpy as np
from contextlib import ExitStack

import concourse.bacc as bacc
import concourse.tile as tile
from concourse import mybir
from concourse.bass_utils import run_bass_kernel_spmd

# Problem shapes (hardcoded per contest contract)
B, S, H = 16, 4096, 1024
NCORES = 8
B_LOC = B // NCORES            # 2 batches per core
ROWS = B_LOC * S               # 8192 rows of enc per core
P = 128
N_TILES = ROWS // P            # 64 tiles of [128, 1024]
TILES_PER_CHUNK = 4            # max DMA chunk = [128, 4, 1024] = 2 MiB
TILES_PER_BATCH = S // P       # 32 score columns per batch
# chunk schedule: (kind, ntiles); 'A' = fp32 fused-STT chunks (DVE),
# 'B' = fp16 cast-DMA chunks (DVE mult + ACT accum). A chunks are spread
# out so DVE/ACT load stays balanced over time; small final chunks trim
# the kernel tail.
CHUNKS = []
for ci in range(15):
    CHUNKS.append(('A' if ci in (2, 4, 7, 9, 12, 14) else 'B', 4))
# tail: finish on an A chunk so the ACT engine (which trails the DVE by one
# product in the B pipeline) is not the last engine standing
CHUNKS += [('B', 2), ('A', 2)]
A_BUFS = 4
B_BUFS = 10

F32 = mybir.dt.float32
F16 = mybir.dt.float16

# set by test.py to capture a profile; harness leaves these untouched
TRACE = False
TMPDIR = None
LAST_RESULT = None


def _softmax_batch(nc, b, scores, smalls, psum_sm, identity, ones_pp, neg_c,
                   out_ap):
    """Softmax over one batch's [128, 32] score block + store to HBM.

    exp(score - C) with the host-chosen constant shift C, per-partition row
    sums from the activation's accum_out, then one ones-matmul that both
    sums across partitions and broadcasts the total."""
    sb = scores[:, b * TILES_PER_BATCH : (b + 1) * TILES_PER_BATCH]
    pexp = smalls.tile([P, TILES_PER_BATCH], F32, tag=f"pexp_{b}")
    s1 = smalls.tile([P, 1], F32, tag=f"s1_{b}")
    nc.scalar.activation(out=pexp, in_=sb,
                         func=mybir.ActivationFunctionType.Exp,
                         bias=neg_c, scale=1.0, accum_out=s1)
    # total sum across partitions, broadcast to all: ones_pp.T @ s1 -> [128,1];
    # transpose the unnormalized exps in parallel on the PE, then one
    # PSUM-source tensor_scalar fuses the 1/S scale with the PSUM->SBUF copy
    p_S = psum_sm.tile([P, 1], F32, tag="sm")
    nc.tensor.matmul(p_S, lhsT=ones_pp, rhs=s1, start=True, stop=True)
    p_yt = psum_sm.tile([TILES_PER_BATCH, P], F32, tag="smt")
    nc.tensor.transpose(p_yt, pexp, identity)
    rb = smalls.tile([TILES_PER_BATCH, 1], F32, tag=f"rb_{b}")
    nc.vector.reciprocal(out=rb, in_=p_S[0:TILES_PER_BATCH, :])
    yt = smalls.tile([TILES_PER_BATCH, P], F32, tag=f"yt_{b}")
    nc.vector.tensor_scalar_mul(out=yt, in0=p_yt, scalar1=rb)
    # batch 0 stores mid-stream via SWDGE (keeps the HWDGE rings FIFO-clean
    # for enc); batch 1 is the kernel tail — use the by-then-idle SP ring,
    # whose HWDGE descriptor path is ~0.5us faster than SWDGE
    eng = nc.gpsimd if b == 0 else nc.sync
    eng.dma_start(out=out_ap[b, 0, :].rearrange("(t p) -> t p", p=P), in_=yt)


def _emit(ctx: ExitStack, tc: tile.TileContext, enc_h, u_h, u16_h, c_h, out_h):
    nc = tc.nc
    enc_ap = enc_h[:, :, :]
    u_ap = u_h[:, :]
    out_ap = out_h[:, :, :]

    singles = ctx.enter_context(tc.tile_pool(name="singles", bufs=1))
    ch32s = ctx.enter_context(tc.tile_pool(name="ch32s", bufs=A_BUFS))
    ch16s = ctx.enter_context(tc.tile_pool(name="ch16s", bufs=B_BUFS))
    prods = ctx.enter_context(tc.tile_pool(name="prods", bufs=4))
    smalls = ctx.enter_context(tc.tile_pool(name="smalls", bufs=1))
    psum_sm = ctx.enter_context(tc.tile_pool(name="psum_sm", bufs=1, space="PSUM"))

    # constants; the bootstrap loads (u/c pre-broadcast by the host, then
    # identity) ride the SP HWDGE ring FIRST — ahead of the enc chunks
    # queued behind them — so compute can start as soon as the first enc
    # tile lands; the SWDGE queue starts streaming fp16 chunks at t=0
    ones_1p = singles.tile([1, P], F32)
    nc.vector.memset(ones_1p, 1.0)
    ones_pp = singles.tile([P, P], F32)
    nc.vector.memset(ones_pp, 1.0)

    # u/c arrive already broadcast across partitions ([128,H] / [128,1]);
    # no PE ones-matmul chain on the critical path
    u_bcast16 = singles.tile([P, H], F16)
    nc.sync.dma_start(out=u_bcast16, in_=u16_h[:, :])
    u_bcast = singles.tile([P, H], F32)
    nc.sync.dma_start(out=u_bcast, in_=u_ap)
    neg_c = singles.tile([P, 1], F32)
    nc.sync.dma_start(out=neg_c, in_=c_h[:, :])
    id_dram = nc.inline_tensor(np.eye(P, dtype=np.float32), name="id128")
    identity = singles.tile([P, P], F32)
    nc.sync.dma_start(out=identity, in_=id_dram[:, :])

    # warm the ACT exp table set early so the mid-stream softmax doesn't
    # stall ACT behind a ~2.7us ACT_TABLE_LOAD
    warm = smalls.tile([1, 2], F32, tag="warm")
    nc.scalar.activation(out=warm, in_=ones_1p[:, 0:2],
                         func=mybir.ActivationFunctionType.Exp)

    # ---- main loop: scores[r] = enc_row[r] . u ----------------------------
    scores = singles.tile([P, N_TILES], F32)   # col, row p -> flat row col*128+p
    scratch_v = singles.tile([P, H], F32)      # STT mandatory full-product dump
    scratch_a = singles.tile([P, H], F16)      # ACT activation mandatory out
    enc_flat = enc_ap.flatten_outer_dims()     # [8192, 1024]
    col0 = 0
    n_hw = 0
    for kind, nt in CHUNKS:
        # one dma_start per 512 KiB tile (not per 2 MiB chunk): with three
        # queues sharing the SDMA engines round-robin, a whole-chunk DMA has
        # ~17us completion latency and whole-chunk sems would stall compute
        # that long; per-tile sems cut the pipeline fill/drain latency 4x
        if kind == 'A':
            ch = ch32s.tile([P, TILES_PER_CHUNK, H], F32, tag="c32")
            # all fp32 tile DMAs trigger from the otherwise-idle SP ring:
            # triggers on the ACT sequencer get stuck behind its accumulate
            # backlog and starve that ring (benched 11us slower)
            eng = nc.sync
            n_hw += 1
            for t in range(nt):
                col = col0 + t
                src = enc_flat[col * P : (col + 1) * P, :]
                eng.dma_start(out=ch[:, t, :], in_=src)
                # fused multiply+row-sum: out=(in0*1.0)*in1, accum=sum(out)
                nc.vector.scalar_tensor_tensor(
                    out=scratch_v,
                    in0=ch[:, t, :],
                    scalar=1.0,
                    in1=u_bcast,
                    op0=mybir.AluOpType.mult,
                    op1=mybir.AluOpType.mult,
                    accum_out=scores[:, col : col + 1],
                )
        else:
            ch = ch16s.tile([P, TILES_PER_CHUNK, H], F16, tag="c16")
            for t in range(nt):
                col = col0 + t
                src = enc_flat[col * P : (col + 1) * P, :]
                nc.gpsimd.dma_start(out=ch[:, t, :], in_=src)  # f32->f16 cast
                prod = prods.tile([P, H], F16, tag="prod")
                nc.vector.tensor_tensor(out=prod, in0=ch[:, t, :],
                                        in1=u_bcast16,
                                        op=mybir.AluOpType.mult)
                nc.scalar.activation(out=scratch_a, in_=prod,
                                     func=mybir.ActivationFunctionType.Copy,
                                     accum_out=scores[:, col : col + 1])
        col0 += nt
        # softmax for a batch as soon as its 32 score columns are done
        if col0 == TILES_PER_BATCH:
            _softmax_batch(nc, 0, scores, smalls, psum_sm, identity, ones_pp,
                           neg_c, out_ap)
        elif col0 == N_TILES:
            _softmax_batch(nc, 1, scores, smalls, psum_sm, identity, ones_pp,
                           neg_c, out_ap)


def build_bass():
    nc = bacc.Bacc("TRN2", target_bir_lowering=False)
    enc_h = nc.dram_tensor("enc", [B_LOC, S, H], F32, kind="ExternalInput")
    u_h = nc.dram_tensor("u", [P, H], F32, kind="ExternalInput")
    u16_h = nc.dram_tensor("u16", [P, H], F16, kind="ExternalInput")
    c_h = nc.dram_tensor("c", [P, 1], F32, kind="ExternalInput")
    out_h = nc.dram_tensor("out", [B_LOC, 1, S], F32, kind="ExternalOutput")
    with ExitStack() as ctx:
        tc = ctx.enter_context(tile.TileContext(nc))
        _emit(ctx, tc, enc_h, u_h, u16_h, c_h, out_h)
    nc.compile()
    return nc


_NC = None


def _get_nc():
    global _NC
    if _NC is None:
        _NC = build_bass()
    return _NC


def kernel(hidden, encoder_outputs, W, b, v):
    global LAST_RESULT
    nc = _get_nc()
    we = np.asarray(W, dtype=np.float32)[:, H:]
    v2 = np.asarray(v, dtype=np.float32)
    # u = v @ We on the host (1M MACs of input prep; the O(B*S*H) work all
    # happens on-device)
    u = (v2[0].astype(np.float64) @ we.astype(np.float64)).astype(np.float32)
    # shift constant: exp(max - C) can't overflow (needs max > C + 88,
    # ~8 sigma) and can't all-underflow (needs max < C - 88 < 0.6 sigma)
    c = np.float32(4.5) * np.float32(np.linalg.norm(u.astype(np.float64)))
    u2 = np.ascontiguousarray(np.broadcast_to(u.reshape(1, H), (P, H)))
    u16 = np.ascontiguousarray(u2.astype(np.float16))
    negc = np.full((P, 1), -c, dtype=np.float32)
    enc = np.asarray(encoder_outputs, dtype=np.float32)
    in_maps = [
        {
            "enc": np.ascontiguousarray(enc[i * B_LOC : (i + 1) * B_LOC]),
            "u": u2,
            "u16": u16,
            "c": negc,
        }
        for i in range(NCORES)
    ]
    res = run_bass_kernel_spmd(nc, in_maps, core_ids=list(range(NCORES)),
                               trace=TRACE, tmpdir=TMPDIR)
    LAST_RESULT = res
    return np.concatenate([res.results[i]["out"] for i in range(NCORES)], axis=0)


# revision 23
# speedup vs baseline: 1.0668x; 1.0668x over previous
"""Trainium2 Bass kernel for nn_Attn_32925219291574.

Math: reference computes softmax_s( v . (W @ [hidden; enc[b,s]] + b) ).
Split W = [Wh | We]. The hidden/bias part v.(Wh@hidden + b) is constant in s,
and softmax is shift-invariant, so the output is exactly
    softmax_s( enc[b,s,:] . u ),   u = v @ We    (We = W[:, H:2H])
`hidden` and `b` never affect the output. u (4 KB) is computed on the host
during input sharding, so the kernel is a pure stream over the 256 MiB
encoder_outputs tensor: per-row dot products, then a softmax per batch.

Engine budget: the fused multiply+row-sum (TensorScalarPtr/accum_out) runs
only in the DVE's 1x perf mode (~1.5us per [128,1024] fp32 tile -> ~96us for
all 64 tiles, which made DVE the baseline's bottleneck, above the ~94us HBM
streaming floor). So the work is split into two pipelines:
  A (20 tiles, fp32): fused STT on DVE, exact.
  B (44 tiles, fp16): chunk is cast f32->fp16 during the DMA (SWDGE/gpsimd
     queue, the only engine that can cast), DVE does a plain tensor_tensor
     multiply (2x_1p mode, ~0.9us), and the ACT engine row-sums the product
     via activation(Copy, accum_out) (~1.5us) in parallel.
fp16 quantization of enc/u perturbs the logits by ~0.01 (measured softmax
rel err ~1e-3, budget 2e-2). DVE ~72us, ACT ~70us, both under the DMA floor.

The softmax uses a fixed shift C = 4.5*||u|| instead of the data max
(scores ~ N(0, ~1.2||u||) since enc is unit-normal; exp(max-C) can neither
overflow nor all-underflow within ~8 sigma) — this removes the max
reduction/transpose/broadcast from the kernel tail entirely.

Sharding: data-parallel over batch B=16 -> 2 batches per core, no cross-core
communication. fp32 chunks stream on the two HWDGE rings (SP/ACT); fp16
chunks + u load + output stores ride the SWDGE (gpsimd) queue.
"""

import numpy as np
from contextlib import ExitStack

import concourse.bacc as bacc
import concourse.tile as tile
from concourse import mybir
from concourse.bass_utils import run_bass_kernel_spmd

# Problem shapes (hardcoded per contest contract)
B, S, H = 16, 4096, 1024
NCORES = 8
B_LOC = B // NCORES            # 2 batches per core
ROWS = B_LOC * S               # 8192 rows of enc per core
P = 128
N_TILES = ROWS // P            # 64 tiles of [128, 1024]
TILES_PER_CHUNK = 4            # max DMA chunk = [128, 4, 1024] = 2 MiB
TILES_PER_BATCH = S // P       # 32 score columns per batch
# chunk schedule: (kind, ntiles); 'A' = fp32 fused-STT chunks (DVE),
# 'B' = fp16 cast-DMA chunks (DVE mult + ACT accum). A chunks are spread
# out so DVE/ACT load stays balanced over time; small final chunks trim
# the kernel tail.
CHUNKS = []
for ci in range(15):
    CHUNKS.append(('A' if ci in (2, 4, 7, 9, 12, 14) else 'B', 4))
# tail: finish on an A chunk so the ACT engine (which trails the DVE by one
# product in the B pipeline) is not the last engine standing
CHUNKS += [('B', 2), ('A', 2)]
A_BUFS = 6
B_BUFS = 9

F32 = mybir.dt.float32
F16 = mybir.dt.float16

# set by test.py to capture a profile; harness leaves these untouched
TRACE = False
TMPDIR = None
LAST_RESULT = None


def _softmax_batch(nc, b, scores, smalls, psum_sm, identity, ones_pp, neg_c,
                   out_ap):
    """Softmax over one batch's [128, 32] score block + store to HBM.

    exp(score - C) with the host-chosen constant shift C, per-partition row
    sums from the activation's accum_out, then one ones-matmul that both
    sums across partitions and broadcasts the total."""
    sb = scores[:, b * TILES_PER_BATCH : (b + 1) * TILES_PER_BATCH]
    pexp = smalls.tile([P, TILES_PER_BATCH], F32, tag=f"pexp_{b}")
    s1 = smalls.tile([P, 1], F32, tag=f"s1_{b}")
    nc.scalar.activation(out=pexp, in_=sb,
                         func=mybir.ActivationFunctionType.Exp,
                         bias=neg_c, scale=1.0, accum_out=s1)
    # total sum across partitions, broadcast to all: ones_pp.T @ s1 -> [128,1];
    # transpose the unnormalized exps in parallel on the PE, then one
    # PSUM-source tensor_scalar fuses the 1/S scale with the PSUM->SBUF copy
    p_S = psum_sm.tile([P, 1], F32, tag="sm")
    nc.tensor.matmul(p_S, lhsT=ones_pp, rhs=s1, start=True, stop=True)
    p_yt = psum_sm.tile([TILES_PER_BATCH, P], F32, tag="smt")
    nc.tensor.transpose(p_yt, pexp, identity)
    rb = smalls.tile([TILES_PER_BATCH, 1], F32, tag=f"rb_{b}")
    nc.vector.reciprocal(out=rb, in_=p_S[0:TILES_PER_BATCH, :])
    yt = smalls.tile([TILES_PER_BATCH, P], F32, tag=f"yt_{b}")
    nc.vector.tensor_scalar_mul(out=yt, in0=p_yt, scalar1=rb)
    # batch 0 stores mid-stream via SWDGE (keeps the HWDGE rings FIFO-clean
    # for enc); batch 1 is the kernel tail — use the by-then-idle SP ring,
    # whose HWDGE descriptor path is ~0.5us faster than SWDGE
    eng = nc.gpsimd if b == 0 else nc.sync
    eng.dma_start(out=out_ap[b, 0, :].rearrange("(t p) -> t p", p=P), in_=yt)


def _emit(ctx: ExitStack, tc: tile.TileContext, enc_h, u_h, u16_h, c_h, out_h):
    nc = tc.nc
    enc_ap = enc_h[:, :, :]
    u_ap = u_h[:, :]
    out_ap = out_h[:, :, :]

    singles = ctx.enter_context(tc.tile_pool(name="singles", bufs=1))
    ch32s = ctx.enter_context(tc.tile_pool(name="ch32s", bufs=A_BUFS))
    ch16s = ctx.enter_context(tc.tile_pool(name="ch16s", bufs=B_BUFS))
    prods = ctx.enter_context(tc.tile_pool(name="prods", bufs=4))
    smalls = ctx.enter_context(tc.tile_pool(name="smalls", bufs=1))
    psum_sm = ctx.enter_context(tc.tile_pool(name="psum_sm", bufs=1, space="PSUM"))

    # constants; the bootstrap loads (u/c pre-broadcast by the host, then
    # identity) ride the SP HWDGE ring FIRST — ahead of the enc chunks
    # queued behind them — so compute can start as soon as the first enc
    # tile lands; the SWDGE queue starts streaming fp16 chunks at t=0
    ones_1p = singles.tile([1, P], F32)
    nc.vector.memset(ones_1p, 1.0)
    ones_pp = singles.tile([P, P], F32)
    nc.vector.memset(ones_pp, 1.0)

    # u/c arrive already broadcast across partitions ([128,H] / [128,1]);
    # no PE ones-matmul chain on the critical path. u16 (gates the first
    # TT) and u (gates the first STT) load on different rings in parallel.
    u_bcast16 = singles.tile([P, H], F16)
    nc.sync.dma_start(out=u_bcast16, in_=u16_h[:, :])
    u_bcast = singles.tile([P, H], F32)
    nc.scalar.dma_start(out=u_bcast, in_=u_ap)
    neg_c = singles.tile([P, 1], F32)
    nc.sync.dma_start(out=neg_c, in_=c_h[:, :])
    id_dram = nc.inline_tensor(np.eye(P, dtype=np.float32), name="id128")
    identity = singles.tile([P, P], F32)
    nc.sync.dma_start(out=identity, in_=id_dram[:, :])

    # warm the ACT exp table set early so the mid-stream softmax doesn't
    # stall ACT behind a ~2.7us ACT_TABLE_LOAD
    warm = smalls.tile([1, 2], F32, tag="warm")
    nc.scalar.activation(out=warm, in_=ones_1p[:, 0:2],
                         func=mybir.ActivationFunctionType.Exp)

    # ---- main loop: scores[r] = enc_row[r] . u ----------------------------
    scores = singles.tile([P, N_TILES], F32)   # col, row p -> flat row col*128+p
    scratch_v = singles.tile([P, H], F32)      # STT mandatory full-product dump
    scratch_a = singles.tile([P, H], F16)      # ACT activation mandatory out
    enc_flat = enc_ap.flatten_outer_dims()     # [8192, 1024]
    col0 = 0
    n_hw = 0
    for kind, nt in CHUNKS:
        # one dma_start per 512 KiB tile (not per 2 MiB chunk): with three
        # queues sharing the SDMA engines round-robin, a whole-chunk DMA has
        # ~17us completion latency and whole-chunk sems would stall compute
        # that long; per-tile sems cut the pipeline fill/drain latency 4x
        if kind == 'A':
            ch = ch32s.tile([P, TILES_PER_CHUNK, H], F32, tag="c32")
            # alternate the two HWDGE rings per chunk; A_BUFS covers 6 chunks
            # so the triggers (incl. the ones on the busy ACT sequencer)
            # never wait on a buffer slot and the rings stay fed
            eng = nc.sync if n_hw % 2 == 0 else nc.scalar
            n_hw += 1
            for t in range(nt):
                col = col0 + t
                src = enc_flat[col * P : (col + 1) * P, :]
                eng.dma_start(out=ch[:, t, :], in_=src)
                # fused multiply+row-sum: out=(in0*1.0)*in1, accum=sum(out)
                nc.vector.scalar_tensor_tensor(
                    out=scratch_v,
                    in0=ch[:, t, :],
                    scalar=1.0,
                    in1=u_bcast,
                    op0=mybir.AluOpType.mult,
                    op1=mybir.AluOpType.mult,
                    accum_out=scores[:, col : col + 1],
                )
        else:
            ch = ch16s.tile([P, TILES_PER_CHUNK, H], F16, tag="c16")
            for t in range(nt):
                col = col0 + t
                src = enc_flat[col * P : (col + 1) * P, :]
                nc.gpsimd.dma_start(out=ch[:, t, :], in_=src)  # f32->f16 cast
                prod = prods.tile([P, H], F16, tag="prod")
                nc.vector.tensor_tensor(out=prod, in0=ch[:, t, :],
                                        in1=u_bcast16,
                                        op=mybir.AluOpType.mult)
                nc.scalar.activation(out=scratch_a, in_=prod,
                                     func=mybir.ActivationFunctionType.Copy,
                                     accum_out=scores[:, col : col + 1])
        col0 += nt
        # softmax for a batch as soon as its 32 score columns are done
        if col0 == TILES_PER_BATCH:
            _softmax_batch(nc, 0, scores, smalls, psum_sm, identity, ones_pp,
                           neg_c, out_ap)
        elif col0 == N_TILES:
            _softmax_batch(nc, 1, scores, smalls, psum_sm, identity, ones_pp,
                           neg_c, out_ap)


def build_bass():
    nc = bacc.Bacc("TRN2", target_bir_lowering=False)
    enc_h = nc.dram_tensor("enc", [B_LOC, S, H], F32, kind="ExternalInput")
    u_h = nc.dram_tensor("u", [P, H], F32, kind="ExternalInput")
    u16_h = nc.dram_tensor("u16", [P, H], F16, kind="ExternalInput")
    c_h = nc.dram_tensor("c", [P, 1], F32, kind="ExternalInput")
    out_h = nc.dram_tensor("out", [B_LOC, 1, S], F32, kind="ExternalOutput")
    with ExitStack() as ctx:
        tc = ctx.enter_context(tile.TileContext(nc))
        _emit(ctx, tc, enc_h, u_h, u16_h, c_h, out_h)
    nc.compile()
    return nc


_NC = None


def _get_nc():
    global _NC
    if _NC is None:
        _NC = build_bass()
    return _NC


def kernel(hidden, encoder_outputs, W, b, v):
    global LAST_RESULT
    nc = _get_nc()
    we = np.asarray(W, dtype=np.float32)[:, H:]
    v2 = np.asarray(v, dtype=np.float32)
    # u = v @ We on the host (1M MACs of input prep; the O(B*S*H) work all
    # happens on-device)
    u = (v2[0].astype(np.float64) @ we.astype(np.float64)).astype(np.float32)
    # shift constant: exp(max - C) can't overflow (needs max > C + 88,
    # ~8 sigma) and can't all-underflow (needs max < C - 88 < 0.6 sigma)
    c = np.float32(4.5) * np.float32(np.linalg.norm(u.astype(np.float64)))
    u2 = np.ascontiguousarray(np.broadcast_to(u.reshape(1, H), (P, H)))
    u16 = np.ascontiguousarray(u2.astype(np.float16))
    negc = np.full((P, 1), -c, dtype=np.float32)
    enc = np.asarray(encoder_outputs, dtype=np.float32)
    in_maps = [
        {
            "enc": np.ascontiguousarray(enc[i * B_LOC : (i + 1) * B_LOC]),
            "u": u2,
            "u16": u16,
            "c": negc,
        }
        for i in range(NCORES)
    ]
    res = run_bass_kernel_spmd(nc, in_maps, core_ids=list(range(NCORES)),
                               trace=TRACE, tmpdir=TMPDIR)
    LAST_RESULT = res
    return np.concatenate([res.results[i]["out"] for i in range(NCORES)], axis=0)
